# revision 56
# baseline (speedup 1.0000x reference)
"""Trainium2 Bass kernel for fused sparse-attention block (nn_Attention_790273982568).

Full (unsharded) inputs in, full output out. Internally: tensor-parallel over
heads across 8 NeuronCores — each core owns 4 Q heads + 1 KV head (wqkv rows)
and 512 output columns of wo (rows of wo), with per-head on-device AllGathers
of the attention outputs (overlapped with attention) before the output
projection.
"""

import os
import sys

import numpy as np

for _p in ("/opt/trn_rl_repo", "/root/.axon_site/_ro/trn_rl_repo"):
    if _p not in sys.path and os.path.isdir(_p):
        sys.path.append(_p)

import ml_dtypes  # noqa: E402

import bass_rust as _bass_rust  # noqa: E402
import concourse.bass as bass  # noqa: E402
from concourse import bacc  # noqa: E402
import concourse.mybir as mybir  # noqa: E402
import concourse.tile as tile  # noqa: E402
from concourse.bass import ds, ts  # noqa: E402
from concourse.bass_utils import run_bass_kernel_spmd  # noqa: E402

# Problem shapes (hardcoded per spec)
T = 2048
DIM = 4096
HD = 128
NH = 32
NKV = 8
NCORES = 8
QH = NH // NCORES          # 4 q heads per core
FEAT = (QH + 2) * HD       # 768 qkv features per core
OUTC = DIM // NCORES       # 512 output columns per core
P = 128
NT = T // P                # 16 token tiles
KC = DIM // P              # 32 contraction chunks
QSPAN = 512
NQS = T // QSPAN           # 4 q spans
HALF = HD // 2
EPS = 1e-5
THETA = 10000.0
SCALE = 1.0 / float(np.sqrt(HD))

BF16 = mybir.dt.bfloat16
F32 = mybir.dt.float32
FP8 = mybir.dt.float8e4
DR = mybir.MatmulPerfMode.DoubleRow
SX = 4.0
SW = 128.0
QKV_DESCALE = SX * SW
AX = mybir.AxisListType
ALU = mybir.AluOpType
ACTF = mybir.ActivationFunctionType

_PROGRAM_CACHE = {}


def _build_body(nc, aps):
    xT8 = aps["xT8"]
    wqkvT8 = aps["wqkvT8"]
    woT = aps["woT"]
    ropeP = aps["ropeP"]
    lnwb = aps["lnwb"]
    masks = aps["masks"]
    ident = aps["ident"]
    ag_in = aps["ag_in"]
    ag_out = aps["ag_out"]       # [QH, NCORES*P, T]
    outT = aps["outT"]
    tc = aps["tc"]

    with tc.tile_pool(name="consts", bufs=1) as consts:
        ident_sb = consts.tile([P, P], BF16, tag="ident")
        nc.sync.dma_start(ident_sb[:], ident[:, :])
        masks_sb = consts.tile([P, 4, QSPAN], BF16, tag="masks")

        # persistent activation strips
        qkT = consts.tile([P, QH + 1, T], BF16, tag="qkT")       # [hd, head, tok]
        vaug = consts.tile([P, NT, HD + 1], BF16, tag="vaug")    # [ktok%, ktile, hd+1]
        nc.vector.memset(vaug[:, :, HD : HD + 1], 1.0)

        # ---------------- Phase 1: QKV projection + LN + RoPE ----------------
        with (
            tc.tile_pool(name="wq", bufs=1) as wq_pool,
            tc.tile_pool(name="p1", bufs=3) as p1,
            tc.tile_pool(name="px", bufs=3) as px,
            tc.tile_pool(name="p1s", bufs=4) as p1s,
            tc.tile_pool(name="prq", bufs=12) as prq_pool,
            tc.tile_pool(name="psum1", bufs=2, space="PSUM") as psum1,
            tc.tile_pool(name="psumT", bufs=2, space="PSUM") as psumT,
        ):
            def load_xt(t, eng=None):
                # two token tiles (2t, 2t+1), pre-tiled fp8 hi/lo
                tiles = px.tile([P, 2, 2, KC, P], FP8, tag="xt", name=f"xt_{t}")
                for tt in range(2):
                    for s in (1, 0):
                        (eng or nc.sync).dma_start(
                            tiles[:, tt, s, :, :], xT8[2 * t + tt, s]
                        )
                return tiles

            wqkvT_sb = wq_pool.tile([P, KC, 2, FEAT], FP8, tag="wqkvT8")

            def load_wq(s, g, eng):
                eng.dma_start(
                    wqkvT_sb[:, ds(8 * g, 8), s, :],
                    wqkvT8[ds(8 * g * P, 8 * P), s, :].rearrange(
                        "(k p) f -> p k f", p=P
                    ),
                )

            # startup order: first matmul needs xt0[tile0].hi + wqkv.hi[g0],
            # so land those first; lo slots and later tiles stream behind
            xt0 = px.tile([P, 2, 2, KC, P], FP8, tag="xt", name="xt_0")
            nc.sync.dma_start(xt0[:, 0, 1, :, :], xT8[0, 1])
            load_wq(0, 0, nc.sync)
            load_wq(0, 1, nc.scalar)
            nc.sync.dma_start(xt0[:, 0, 0, :, :], xT8[0, 0])
            rope_early = True
            load_wq(0, 2, nc.scalar)
            nc.sync.dma_start(xt0[:, 1, 1, :, :], xT8[1, 1])
            load_wq(0, 3, nc.scalar)
            nc.sync.dma_start(xt0[:, 1, 0, :, :], xT8[1, 0])
            for g in range(4):
                load_wq(1, g, nc.scalar if g % 2 else nc.sync)
            rope_sb = wq_pool.tile([P, NT, 2, HALF], F32, tag="rope")
            nc.scalar.dma_start(rope_sb[:], ropeP[:, :, :, :])
            xt_cache = {0: xt0, 1: load_xt(1, nc.scalar)}
            wb_sb = wq_pool.tile([P, 2, 2, HD], F32, tag="wb")
            nc.sync.dma_start(wb_sb[:], lnwb[:, :, :, :])
            nc.sync.dma_start(masks_sb[:], masks[:, :, :])

            rq_pend = []
            for t in range(NT):
                if t % 2 == 0:
                    xt_tiles = xt_cache.pop(t // 2) if t // 2 in xt_cache \
                        else load_xt(t // 2)
                    if t + 4 < NT:
                        xt_cache[t // 2 + 2] = load_xt(
                            t // 2 + 2, nc.scalar if t % 4 else nc.sync)
                sub = t % 2
                pq = psum1.tile([P, FEAT], F32, tag="pqkv")
                for cw in ((0, 512), (512, 256)):
                    for ks in range(KC // 2):
                        nc.tensor.matmul(
                            pq[:, ds(cw[0], cw[1])],
                            xt_tiles[:, sub, 1, ds(2 * ks, 2), :],
                            wqkvT_sb[:, ds(2 * ks, 2), 0, ds(cw[0], cw[1])],
                            start=(ks == 0), stop=False, perf_mode=DR,
                        )
                    for k in range(KC):
                        nc.tensor.matmul(
                            pq[:, ds(cw[0], cw[1])],
                            xt_tiles[:, sub, :, k, :],
                            wqkvT_sb[:, k, :, ds(cw[0], cw[1])],
                            start=False, stop=(k == KC - 1), perf_mode=DR,
                        )
                # v slice straight to vaug (scale back from 512x, exact /2^9)
                nc.vector.tensor_scalar_mul(
                    vaug[:, t, 0:HD], pq[:, 640:FEAT], 1.0 / QKV_DESCALE)
                # q/k slices as bf16 (match reference's bf16 xqkv)
                xq = p1.tile([P, 5 * HD], BF16, tag="xq")
                nc.scalar.copy(xq[:], pq[:, 0 : 5 * HD])

                for h in range(5):  # 4 q heads then k head
                    qk = 0 if h < QH else 1
                    xh = xq[:, ds(h * HD, HD)]
                    s1 = p1s.tile([P, 1], F32, tag="s1")
                    sd = p1s.tile([P, HD], F32, tag="sd")
                    nc.scalar.activation(
                        sd[:], xh, ACTF.Copy, accum_out=s1[:]
                    )
                    sqs = p1s.tile([P, HD], F32, tag="sqs")
                    ssq = p1s.tile([P, 1], F32, tag="ssq")
                    nc.scalar.activation(
                        sqs[:], xh, ACTF.Square, accum_out=ssq[:]
                    )
                    negmu = p1s.tile([P, 1], F32, tag="negmu")
                    nc.vector.tensor_scalar_mul(negmu[:], s1[:], -1.0 / HD)
                    mu2 = p1s.tile([P, 1], F32, tag="mu2")
                    nc.vector.tensor_mul(mu2[:], negmu[:], negmu[:])
                    sqbias = p1s.tile([P, 1], F32, tag="sqbias")
                    nc.vector.tensor_scalar(
                        sqbias[:], mu2[:], -1.0, EPS, op0=ALU.mult, op1=ALU.add
                    )
                    std = p1s.tile([P, 1], F32, tag="std")
                    _si = nc.scalar.activation(
                        std[:], ssq[:], ACTF.Sqrt, bias=sqbias[:], scale=1.0 / HD
                    )
                    aps["last_sqrt"] = _si.ins
                    rstd = p1s.tile([P, 1], F32, tag="rstd")
                    nc.vector.reciprocal(rstd[:], std[:])
                    nbias = p1s.tile([P, 1], F32, tag="nbias")
                    nc.vector.tensor_mul(nbias[:], negmu[:], rstd[:])
                    xn = p1.tile([P, HD], F32, tag="xn")
                    nc.vector.tensor_scalar(
                        xn[:], xh, rstd[:], nbias[:], op0=ALU.mult, op1=ALU.add
                    )
                    if not aps.get("wb_trivial"):
                        # y = xn * w + b
                        nc.vector.tensor_mul(xn[:], xn[:], wb_sb[:, qk, 0, :])
                        nc.vector.tensor_add(xn[:], xn[:], wb_sb[:, qk, 1, :])
                    # rope: re = ye*cos - yo*sin ; ro = ye*sin + yo*cos
                    cosv = rope_sb[:, t, 0, :]
                    sinv = rope_sb[:, t, 1, :]
                    xr = xn.rearrange("p (f two) -> p two f", two=2)
                    xe = xr[:, 0, :]
                    xo = xr[:, 1, :]
                    ta = p1.tile([P, HALF], F32, tag="ta")
                    tb = p1.tile([P, HALF], F32, tag="tb")
                    rq = prq_pool.tile([P, HD], BF16, tag="rq", name=f"rq_{t}_{h}")
                    rqr = rq.rearrange("p (f two) -> p two f", two=2)
                    nc.vector.tensor_mul(ta[:], xe, cosv)
                    nc.vector.tensor_mul(tb[:], xo, sinv)
                    nc.vector.tensor_sub(rqr[:, 0, :], ta[:], tb[:])
                    nc.vector.tensor_mul(ta[:], xe, sinv)
                    nc.vector.tensor_mul(tb[:], xo, cosv)
                    nc.vector.tensor_add(rqr[:, 1, :], ta[:], tb[:])
                    rq_pend.append((h, t, rq))
                # transposes for the previous token block (gives DVE a full
                # block of slack before PE needs the rope output)
                while len(rq_pend) > 5:
                    ph, pt, prq = rq_pend.pop(0)
                    ptr = psumT.tile([P, P], BF16, tag="ptr")
                    nc.tensor.transpose(ptr[:], prq[:], ident_sb[:])
                    nc.vector.tensor_copy(qkT[:, ph, ts(pt, P)], ptr[:])

            while rq_pend:
                ph, pt, prq = rq_pend.pop(0)
                ptr = psumT.tile([P, P], BF16, tag="ptr")
                nc.tensor.transpose(ptr[:], prq[:], ident_sb[:])
                nc.vector.tensor_copy(qkT[:, ph, ts(pt, P)], ptr[:])

        # ---------------- Phase 2: attention (+ per-head AllGather) ----------
        with (
            tc.tile_pool(name="w3", bufs=1) as w3,
            tc.tile_pool(name="p3", bufs=36) as p3,
            tc.tile_pool(name="p3o", bufs=3) as p3o,
            tc.tile_pool(name="paoT", bufs=2) as paoT,
        ):
            # prefetch wo weights while attention runs
            woT_sb = w3.tile([P, KC, OUTC], BF16, tag="woT")
            for k in range(KC):
                nc.sync.dma_start(woT_sb[:, k, :], woT[ts(k, P), :])

            ao0 = [None] * KC  # first-half ao tiles, prefetched per head
            with (
                tc.tile_pool(name="p2", bufs=2) as p2,
                tc.tile_pool(name="p2s", bufs=4) as p2s,
                tc.tile_pool(name="paob", bufs=12) as paob,
                tc.tile_pool(name="psum_s", bufs=2, space="PSUM") as psum_s_pool,
                tc.tile_pool(name="psum_o", bufs=2, space="PSUM") as psum_o_pool,
                tc.tile_pool(name="psum_t2", bufs=2, space="PSUM") as psum_t2,
            ):
                def emit_scores(h, j):
                    nkb = 4 * (j + 1)
                    attn = p2.tile([P, NT, QSPAN], BF16, tag="attn",
                                   name=f"attn_{h}_{j}")
                    for ip in range(nkb // 2):
                        i = 2 * ip
                        ps = psum_s_pool.tile([P, 2, QSPAN], F32, tag="ps")
                        for u in range(2):
                            nc.tensor.matmul(
                                ps[:, u, :],
                                qkT[:, QH, ts(i + u, P)],
                                qkT[:, h, ds(j * QSPAN, QSPAN)],
                                start=True, stop=True,
                            )
                        # one exp over both blocks (amortize ACT fixed cost)
                        _ei = nc.scalar.activation(
                            attn[:, i : i + 2, :], ps[:], ACTF.Exp, scale=SCALE
                        )
                        if h == 0 and aps.get("last_sqrt") is not None:
                            # keep early exps ordered after the last LN sqrt so
                            # the scheduler can't thrash the ACT table set
                            _bass_rust.add_dep_helper(
                                _ei.ins, aps["last_sqrt"], sync=True,
                                reason="ACT table-set ordering",
                            )
                        r = i - 4 * j
                        if r >= 0:
                            # diagonal pair: one masking mul over both blocks
                            nc.vector.tensor_mul(
                                attn[:, i : i + 2, :],
                                attn[:, i : i + 2, :],
                                masks_sb[:, r : r + 2, :],
                            )
                    return attn

                def emit_pv_mm(h, j, attn):
                    aobs = []
                    for q4 in range(4):
                        qb = 4 * j + q4
                        po = psum_o_pool.tile([P, HD + 1], F32, tag="po")
                        for i in range(qb + 1):
                            nc.tensor.matmul(
                                po[:],
                                attn[:, i, ts(q4, P)],
                                vaug[:, i, :],
                                start=(i == 0), stop=(i == qb),
                            )
                        recip = p2s.tile([P, 1], F32, tag="recip")
                        nc.vector.reciprocal(recip[:], po[:, HD : HD + 1])
                        aob = paob.tile([P, HD], BF16, tag="aob",
                                        name=f"aob_{h}_{qb}")
                        nc.vector.tensor_scalar_mul(aob[:], po[:, 0:HD], recip[:])
                        aobs.append(aob)
                    return aobs

                def emit_tr(h, j, aobs, aoTh):
                    for q4 in range(4):
                        qb = 4 * j + q4
                        pt2 = psum_t2.tile([P, P], BF16, tag="pt2")
                        nc.tensor.transpose(pt2[:], aobs[q4][:], ident_sb[:])
                        nc.vector.tensor_copy(aoTh[:, ts(qb, P)], pt2[:])
                    if j == NQS - 1:
                        nc.sync.dma_start(ag_in[ts(h, P), :], aoTh[:])
                        _post_head(h)

                def _post_head(h):
                    if aps.get("no_collective"):
                        nc.sync.dma_start(
                            ag_out[h].rearrange("(r p) t -> r p t", p=P),
                            ag_in.rearrange("(a p) t -> a p t", p=P)[
                                ds(h, 1)
                            ].to_broadcast([NCORES, P, T]),
                        )
                    else:
                        nc.gpsimd.collective_compute(
                            "AllGather",
                            ALU.bypass,
                            replica_groups=[list(range(NCORES))],
                            ins=[ag_in[ts(h, P), :]],
                            outs=[ag_out[h]],
                        )
                    # prefetch this head's first-half ao tiles for phase 3
                    for r in range(NCORES):
                        k = h * NCORES + r
                        a = p3.tile([P, T // 2], BF16, tag="ao", name=f"ao_0_{k}")
                        nc.sync.dma_start(a[:], ag_out[h, ts(r, P), ds(0, T // 2)])
                        ao0[k] = a

                # software pipeline: scores(j) | pv(j-2) | transpose(j-3)
                from collections import deque

                pv_q = deque()   # (h, j, attn)
                tr_q = deque()   # (h, j, aobs, aoTh)
                aoThs = {}
                spans = [(h, j) for h in range(QH) for j in range(NQS)]

                def step_pv():
                    ph, pj, pattn = pv_q.popleft()
                    tr_q.append((ph, pj, emit_pv_mm(ph, pj, pattn), aoThs[ph]))

                def step_tr():
                    emit_tr(*tr_q.popleft())

                for h, j in spans:
                    if j == 0:
                        aoThs[h] = paoT.tile(
                            [P, T], BF16, tag="aoTh", name=f"aoT_{h}"
                        )
                    attn = emit_scores(h, j)
                    pv_q.append((h, j, attn))
                    if len(pv_q) > 1:
                        step_pv()
                    if len(tr_q) > 1:
                        step_tr()
                while pv_q:
                    step_pv()
                    while len(tr_q) > 1:
                        step_tr()
                while tr_q:
                    step_tr()

            # ---------------- Phase 3: output projection ----------------
            with tc.tile_pool(name="psum3", bufs=8, space="PSUM") as psum3:
                ao1 = [None] * KC
                for th in range(2):  # token halves
                    for cbg in range(2):  # 2 col-block groups -> evac overlap
                        pos = [
                            psum3.tile([P, 512], F32, tag="po3",
                                       name=f"po3_{th}_{cbg}_{i}")
                            for i in range(4)
                        ]
                        for k in range(KC):
                            h, r = divmod(k, NCORES)
                            if th == 0:
                                a = ao0[k]
                            elif cbg == 0:
                                a = p3.tile([P, T // 2], BF16, tag="ao",
                                            name=f"ao_1_{k}")
                                nc.sync.dma_start(
                                    a[:], ag_out[h, ts(r, P), ds(T // 2, T // 2)]
                                )
                                ao1[k] = a
                            else:
                                a = ao1[k]
                            for cc in range(2):
                                cb = cbg * 2 + cc
                                for s2 in range(2):
                                    nc.tensor.matmul(
                                        pos[cc * 2 + s2][:],
                                        woT_sb[:, k, ts(cb, P)],
                                        a[:, ts(s2, 512)],
                                        start=(k == 0), stop=(k == KC - 1),
                                    )
                        for cc in range(2):
                            cb = cbg * 2 + cc
                            for s2 in range(2):
                                ob = p3o.tile(
                                    [P, 512], BF16, tag="ob",
                                    name=f"ob_{th}_{cb}_{s2}"
                                )
                                # split evacuation across DVE and ACT so the
                                # final drain isn't serial on one engine
                                if s2 == 0:
                                    nc.vector.tensor_copy(ob[:], pos[cc * 2 + s2][:])
                                else:
                                    nc.scalar.copy(ob[:], pos[cc * 2 + s2][:])
                                nc.sync.dma_start(
                                    outT[ts(cb, P), ds(th * (T // 2) + s2 * 512, 512)],
                                    ob[:],
                                )


def _build_program(no_collective=False, reps=1, wb_trivial=True):
    nc = bacc.Bacc(
        "TRN2",
        target_bir_lowering=False,
        debug=False,
        enable_asserts=True,
        num_devices=1 if no_collective else NCORES,
    )
    aps = {
        "xT8": nc.dram_tensor(
            "xT8", [NT, 2, P, KC, P], FP8, kind="ExternalInput"
        ).ap(),
        "wqkvT8": nc.dram_tensor(
            "wqkvT8", [DIM, 2, FEAT], FP8, kind="ExternalInput"
        ).ap(),
        "woT": nc.dram_tensor("woT", [NH * HD, OUTC], BF16, kind="ExternalInput").ap(),
        "ropeP": nc.dram_tensor(
            "ropeP", [P, NT, 2, HALF], F32, kind="ExternalInput"
        ).ap(),
        "lnwb": nc.dram_tensor("lnwb", [P, 2, 2, HD], F32, kind="ExternalInput").ap(),
        "masks": nc.dram_tensor("masks", [P, 4, QSPAN], BF16, kind="ExternalInput").ap(),
        "ident": nc.dram_tensor("ident", [P, P], BF16, kind="ExternalInput").ap(),
        "ag_in": nc.dram_tensor("ag_in", [QH * HD, T], BF16).ap(),
        "ag_out": nc.dram_tensor(
            "ag_out", [QH, NCORES * P, T], BF16, addr_space="Shared"
        ).ap(),
        "outT": nc.dram_tensor("outT", [OUTC, T], BF16, kind="ExternalOutput").ap(),
    }
    aps["no_collective"] = no_collective
    aps["wb_trivial"] = wb_trivial
    with tile.TileContext(nc) as tc:
        aps["tc"] = tc
        for _rep in range(reps):
            _build_body(nc, aps)
    nc.compile()
    return nc


def get_program(wb_trivial=True):
    key = ("nc", wb_trivial)
    if key not in _PROGRAM_CACHE:
        _PROGRAM_CACHE[key] = _build_program(wb_trivial=wb_trivial)
    return _PROGRAM_CACHE[key]


def _rope_tables():
    """cos/sin tables computed exactly like the reference (jax fp32 on cpu)."""
    try:
        import jax

        cpu = jax.devices("cpu")[0]
        with jax.default_device(cpu):
            import jax.numpy as jnp

            inv_freq = 1.0 / (
                THETA ** (jnp.arange(HALF, dtype=jnp.float32) * 2.0 / HD)
            )
            pos = jnp.arange(T, dtype=jnp.float32)
            ang = pos[:, None] * inv_freq[None, :]
            cos = np.asarray(jnp.cos(ang), dtype=np.float32)
            sin = np.asarray(jnp.sin(ang), dtype=np.float32)
    except Exception:
        inv_freq = (
            1.0 / (THETA ** (np.arange(HALF, dtype=np.float32) * 2.0 / HD))
        ).astype(np.float32)
        ang = np.arange(T, dtype=np.float32)[:, None] * inv_freq[None, :]
        cos = np.cos(ang).astype(np.float32)
        sin = np.sin(ang).astype(np.float32)
    return cos, sin


def _make_const_inputs(q_ln_w, q_ln_b, k_ln_w, k_ln_b):
    cos, sin = _rope_tables()  # [T, HALF] f32
    ropeP = np.zeros((P, NT, 2, HALF), np.float32)
    ropeP[:, :, 0] = cos.reshape(NT, P, HALF).transpose(1, 0, 2)
    ropeP[:, :, 1] = sin.reshape(NT, P, HALF).transpose(1, 0, 2)

    lnwb = np.zeros((P, 2, 2, HD), np.float32)
    lnwb[:, 0, 0] = np.asarray(q_ln_w, np.float32)[None, :]
    lnwb[:, 0, 1] = np.asarray(q_ln_b, np.float32)[None, :]
    lnwb[:, 1, 0] = np.asarray(k_ln_w, np.float32)[None, :]
    lnwb[:, 1, 1] = np.asarray(k_ln_b, np.float32)[None, :]

    f = np.arange(QSPAN)[None, None, :]
    r = np.arange(4)[None, :, None]
    p = np.arange(P)[:, None, None]
    masks = (f >= 128 * r + p).astype(ml_dtypes.bfloat16)  # [P, 4, QSPAN]
    ident = np.eye(P, dtype=ml_dtypes.bfloat16)
    return ropeP, lnwb, masks, ident


# phase-3 lhsT rows are ordered (h, r, d) = head-of-rank h, rank r; the ao
# feature order is (global head g = 4r+h, d). Permute woT rows to match.
_WOT_PERM = np.empty(NH * HD, np.int64)
for _h in range(QH):
    for _r in range(NCORES):
        _j = (_h * NCORES + _r) * HD
        _g = (4 * _r + _h) * HD
        _WOT_PERM[_j : _j + HD] = np.arange(_g, _g + HD)


def _split8(x):
    hi = x.astype(ml_dtypes.float8_e4m3)
    lo = (x - hi.astype(np.float32)).astype(ml_dtypes.float8_e4m3)
    return hi, lo


def make_in_maps(inputs):
    x = np.asarray(inputs["x"], dtype=ml_dtypes.bfloat16).astype(np.float32)
    wqkv = np.asarray(inputs["wqkv"], dtype=ml_dtypes.bfloat16).astype(np.float32)
    wo = np.asarray(inputs["wo"], dtype=ml_dtypes.bfloat16)
    q_ln_w = np.asarray(inputs["q_ln_w"], np.float32)
    q_ln_b = np.asarray(inputs["q_ln_b"], np.float32)
    k_ln_w = np.asarray(inputs["k_ln_w"], np.float32)
    k_ln_b = np.asarray(inputs["k_ln_b"], np.float32)

    ropeP, lnwb, masks, ident = _make_const_inputs(q_ln_w, q_ln_b, k_ln_w, k_ln_b)
    xh, xl = _split8(SX * x)

    def _tile_x(a):
        return a.reshape(NT, P, KC, P).transpose(0, 3, 2, 1)

    xT8 = np.ascontiguousarray(np.stack([_tile_x(xl), _tile_x(xh)], axis=1))

    in_maps = []
    for c in range(NCORES):
        qrows = wqkv[c * QH * HD : (c + 1) * QH * HD]
        krows = wqkv[NH * HD + c * HD : NH * HD + (c + 1) * HD]
        vrows = wqkv[(NH + NKV) * HD + c * HD : (NH + NKV) * HD + (c + 1) * HD]
        wq_c = np.concatenate([qrows, krows, vrows], axis=0).T
        wh, wl = _split8(SW * wq_c)
        wqkvT8_c = np.ascontiguousarray(np.stack([wh, wl], axis=1))
        woT_c = np.ascontiguousarray(
            wo[c * OUTC : (c + 1) * OUTC, :].T[_WOT_PERM, :]
        )
        in_maps.append(
            {
                "xT8": xT8,
                "wqkvT8": wqkvT8_c,
                "woT": woT_c,
                "ropeP": ropeP,
                "lnwb": lnwb,
                "masks": masks,
                "ident": ident,
            }
        )
    return in_maps


def _wb_trivial(inputs):
    return bool(
        np.all(np.asarray(inputs["q_ln_w"], np.float32) == 1.0)
        and np.all(np.asarray(inputs["k_ln_w"], np.float32) == 1.0)
        and np.all(np.asarray(inputs["q_ln_b"], np.float32) == 0.0)
        and np.all(np.asarray(inputs["k_ln_b"], np.float32) == 0.0)
    )


def kernel(**inputs):
    nc = get_program(wb_trivial=_wb_trivial(inputs))
    in_maps = make_in_maps(inputs)
    res = run_bass_kernel_spmd(nc, in_maps, list(range(NCORES)))
    outT_full = np.concatenate(
        [np.asarray(res.results[c]["outT"]) for c in range(NCORES)], axis=0
    )
    return np.ascontiguousarray(outT_full.T).astype(ml_dtypes.bfloat16)


if __name__ == "__main__":
    nc = get_program()
    print("program built ok")



# revision 57
# speedup vs baseline: 1.0103x; 1.0103x over previous
"""Trainium2 Bass kernel for fused sparse-attention block (nn_Attention_790273982568).

Full (unsharded) inputs in, full output out. Internally: tensor-parallel over
heads across 8 NeuronCores — each core owns 4 Q heads + 1 KV head (wqkv rows)
and 512 output columns of wo (rows of wo), with per-head on-device AllGathers
of the attention outputs (overlapped with attention) before the output
projection.
"""

import os
import sys

import numpy as np

for _p in ("/opt/trn_rl_repo", "/root/.axon_site/_ro/trn_rl_repo"):
    if _p not in sys.path and os.path.isdir(_p):
        sys.path.append(_p)

import ml_dtypes  # noqa: E402

import bass_rust as _bass_rust  # noqa: E402
import concourse.bass as bass  # noqa: E402
from concourse import bacc  # noqa: E402
import concourse.mybir as mybir  # noqa: E402
import concourse.tile as tile  # noqa: E402
from concourse.bass import ds, ts  # noqa: E402
from concourse.bass_utils import run_bass_kernel_spmd  # noqa: E402

# Problem shapes (hardcoded per spec)
T = 2048
DIM = 4096
HD = 128
NH = 32
NKV = 8
NCORES = 8
QH = NH // NCORES          # 4 q heads per core
FEAT = (QH + 2) * HD       # 768 qkv features per core
OUTC = DIM // NCORES       # 512 output columns per core
P = 128
NT = T // P                # 16 token tiles
KC = DIM // P              # 32 contraction chunks
QSPAN = 512
NQS = T // QSPAN           # 4 q spans
HALF = HD // 2
EPS = 1e-5
THETA = 10000.0
SCALE = 1.0 / float(np.sqrt(HD))

BF16 = mybir.dt.bfloat16
F32 = mybir.dt.float32
FP8 = mybir.dt.float8e4
DR = mybir.MatmulPerfMode.DoubleRow
SX = 4.0
SW = 128.0
QKV_DESCALE = SX * SW
AX = mybir.AxisListType
ALU = mybir.AluOpType
ACTF = mybir.ActivationFunctionType

_PROGRAM_CACHE = {}


def _build_body(nc, aps):
    xT8 = aps["xT8"]
    wqkvT8 = aps["wqkvT8"]
    woT = aps["woT"]
    ropeP = aps["ropeP"]
    lnwb = aps["lnwb"]
    masks = aps["masks"]
    ident = aps["ident"]
    ag_in = aps["ag_in"]
    ag_out = aps["ag_out"]       # [QH, NCORES*P, T]
    outT = aps["outT"]
    tc = aps["tc"]

    with tc.tile_pool(name="consts", bufs=1) as consts:
        ident_sb = consts.tile([P, P], BF16, tag="ident")
        nc.sync.dma_start(ident_sb[:], ident[:, :])
        masks_sb = consts.tile([P, 4, QSPAN], BF16, tag="masks")

        # persistent activation strips
        qkT = consts.tile([P, QH + 1, T], BF16, tag="qkT")       # [hd, head, tok]
        vaug = consts.tile([P, NT, HD + 1], BF16, tag="vaug")    # [ktok%, ktile, hd+1]
        nc.vector.memset(vaug[:, :, HD : HD + 1], 1.0)

        # ---------------- Phase 1: QKV projection + LN + RoPE ----------------
        with (
            tc.tile_pool(name="wq", bufs=1) as wq_pool,
            tc.tile_pool(name="p1", bufs=3) as p1,
            tc.tile_pool(name="px", bufs=3) as px,
            tc.tile_pool(name="p1s", bufs=4) as p1s,
            tc.tile_pool(name="prq", bufs=12) as prq_pool,
            tc.tile_pool(name="psum1", bufs=2, space="PSUM") as psum1,
            tc.tile_pool(name="psumT", bufs=2, space="PSUM") as psumT,
        ):
            def load_xt(t, eng=None):
                # two token tiles (2t, 2t+1), pre-tiled fp8 hi/lo
                tiles = px.tile([P, 2, 2, KC, P], FP8, tag="xt", name=f"xt_{t}")
                for tt in range(2):
                    for s in (1, 0):
                        (eng or nc.sync).dma_start(
                            tiles[:, tt, s, :, :], xT8[2 * t + tt, s]
                        )
                return tiles

            wqkvT_sb = wq_pool.tile([P, KC, 2, FEAT], FP8, tag="wqkvT8")

            def load_wq(s, g, eng):
                eng.dma_start(
                    wqkvT_sb[:, ds(8 * g, 8), s, :],
                    wqkvT8[ds(8 * g * P, 8 * P), s, :].rearrange(
                        "(k p) f -> p k f", p=P
                    ),
                )

            # startup order: first matmul needs xt0[tile0].hi + wqkv.hi[g0],
            # so land those first; lo slots and later tiles stream behind
            xt0 = px.tile([P, 2, 2, KC, P], FP8, tag="xt", name="xt_0")
            nc.sync.dma_start(xt0[:, 0, 1, :, :], xT8[0, 1])
            load_wq(0, 0, nc.sync)
            load_wq(0, 1, nc.scalar)
            nc.sync.dma_start(xt0[:, 0, 0, :, :], xT8[0, 0])
            rope_early = True
            load_wq(0, 2, nc.scalar)
            nc.sync.dma_start(xt0[:, 1, 1, :, :], xT8[1, 1])
            load_wq(0, 3, nc.scalar)
            nc.sync.dma_start(xt0[:, 1, 0, :, :], xT8[1, 0])
            for g in range(4):
                load_wq(1, g, nc.scalar if g % 2 else nc.sync)
            rope_sb = wq_pool.tile([P, NT, 2, HALF], F32, tag="rope")
            nc.scalar.dma_start(rope_sb[:], ropeP[:, :, :, :])
            xt_cache = {0: xt0, 1: load_xt(1, nc.scalar)}
            wb_sb = wq_pool.tile([P, 2, 2, HD], F32, tag="wb")
            nc.sync.dma_start(wb_sb[:], lnwb[:, :, :, :])
            nc.sync.dma_start(masks_sb[:], masks[:, :, :])

            rq_pend = []
            for t in range(NT):
                if t % 2 == 0:
                    xt_tiles = xt_cache.pop(t // 2) if t // 2 in xt_cache \
                        else load_xt(t // 2)
                    if t + 4 < NT:
                        xt_cache[t // 2 + 2] = load_xt(
                            t // 2 + 2, nc.scalar if t % 4 else nc.sync)
                sub = t % 2
                pq = psum1.tile([P, FEAT], F32, tag="pqkv")
                for cw in ((0, 512), (512, 256)):
                    for ks in range(KC // 2):
                        nc.tensor.matmul(
                            pq[:, ds(cw[0], cw[1])],
                            xt_tiles[:, sub, 1, ds(2 * ks, 2), :],
                            wqkvT_sb[:, ds(2 * ks, 2), 0, ds(cw[0], cw[1])],
                            start=(ks == 0), stop=False, perf_mode=DR,
                        )
                    for k in range(KC):
                        nc.tensor.matmul(
                            pq[:, ds(cw[0], cw[1])],
                            xt_tiles[:, sub, :, k, :],
                            wqkvT_sb[:, k, :, ds(cw[0], cw[1])],
                            start=False, stop=(k == KC - 1), perf_mode=DR,
                        )
                # v slice straight to vaug (scale back from 512x, exact /2^9)
                nc.vector.tensor_scalar_mul(
                    vaug[:, t, 0:HD], pq[:, 640:FEAT], 1.0 / QKV_DESCALE)
                # q/k slices as bf16 (match reference's bf16 xqkv)
                xq = p1.tile([P, 5 * HD], BF16, tag="xq")
                nc.scalar.copy(xq[:], pq[:, 0 : 5 * HD])

                for h in range(5):  # 4 q heads then k head
                    qk = 0 if h < QH else 1
                    xh = xq[:, ds(h * HD, HD)]
                    s1 = p1s.tile([P, 1], F32, tag="s1")
                    sd = p1s.tile([P, HD], F32, tag="sd")
                    nc.scalar.activation(
                        sd[:], xh, ACTF.Copy, accum_out=s1[:]
                    )
                    sqs = p1s.tile([P, HD], F32, tag="sqs")
                    ssq = p1s.tile([P, 1], F32, tag="ssq")
                    nc.scalar.activation(
                        sqs[:], xh, ACTF.Square, accum_out=ssq[:]
                    )
                    negmu = p1s.tile([P, 1], F32, tag="negmu")
                    nc.vector.tensor_scalar_mul(negmu[:], s1[:], -1.0 / HD)
                    mu2 = p1s.tile([P, 1], F32, tag="mu2")
                    nc.vector.tensor_mul(mu2[:], negmu[:], negmu[:])
                    sqbias = p1s.tile([P, 1], F32, tag="sqbias")
                    nc.vector.tensor_scalar(
                        sqbias[:], mu2[:], -1.0, EPS, op0=ALU.mult, op1=ALU.add
                    )
                    std = p1s.tile([P, 1], F32, tag="std")
                    _si = nc.scalar.activation(
                        std[:], ssq[:], ACTF.Sqrt, bias=sqbias[:], scale=1.0 / HD
                    )
                    aps["last_sqrt"] = _si.ins
                    rstd = p1s.tile([P, 1], F32, tag="rstd")
                    nc.vector.reciprocal(rstd[:], std[:])
                    nbias = p1s.tile([P, 1], F32, tag="nbias")
                    nc.vector.tensor_mul(nbias[:], negmu[:], rstd[:])
                    xn = p1.tile([P, HD], F32, tag="xn")
                    nc.vector.tensor_scalar(
                        xn[:], xh, rstd[:], nbias[:], op0=ALU.mult, op1=ALU.add
                    )
                    if not aps.get("wb_trivial"):
                        # y = xn * w + b
                        nc.vector.tensor_mul(xn[:], xn[:], wb_sb[:, qk, 0, :])
                        nc.vector.tensor_add(xn[:], xn[:], wb_sb[:, qk, 1, :])
                    # rope: re = ye*cos - yo*sin ; ro = ye*sin + yo*cos
                    cosv = rope_sb[:, t, 0, :]
                    sinv = rope_sb[:, t, 1, :]
                    xr = xn.rearrange("p (f two) -> p two f", two=2)
                    xe = xr[:, 0, :]
                    xo = xr[:, 1, :]
                    ta = p1.tile([P, HALF], F32, tag="ta")
                    tb = p1.tile([P, HALF], F32, tag="tb")
                    rq = prq_pool.tile([P, HD], BF16, tag="rq", name=f"rq_{t}_{h}")
                    rqr = rq.rearrange("p (f two) -> p two f", two=2)
                    nc.vector.tensor_mul(ta[:], xe, cosv)
                    nc.vector.tensor_mul(tb[:], xo, sinv)
                    nc.vector.tensor_sub(rqr[:, 0, :], ta[:], tb[:])
                    nc.vector.tensor_mul(ta[:], xe, sinv)
                    nc.vector.tensor_mul(tb[:], xo, cosv)
                    nc.vector.tensor_add(rqr[:, 1, :], ta[:], tb[:])
                    rq_pend.append((h, t, rq))
                # transposes for the previous token block (gives DVE a full
                # block of slack before PE needs the rope output)
                while len(rq_pend) > 5:
                    ph, pt, prq = rq_pend.pop(0)
                    ptr = psumT.tile([P, P], BF16, tag="ptr")
                    nc.tensor.transpose(ptr[:], prq[:], ident_sb[:])
                    nc.vector.tensor_copy(qkT[:, ph, ts(pt, P)], ptr[:])

            while rq_pend:
                ph, pt, prq = rq_pend.pop(0)
                ptr = psumT.tile([P, P], BF16, tag="ptr")
                nc.tensor.transpose(ptr[:], prq[:], ident_sb[:])
                nc.vector.tensor_copy(qkT[:, ph, ts(pt, P)], ptr[:])

        # ---------------- Phase 2: attention (+ per-head AllGather) ----------
        with (
            tc.tile_pool(name="w3", bufs=1) as w3,
            tc.tile_pool(name="p3", bufs=36) as p3,
            tc.tile_pool(name="p3o", bufs=3) as p3o,
            tc.tile_pool(name="paoT", bufs=2) as paoT,
        ):
            # prefetch wo weights while attention runs
            woT_sb = w3.tile([P, KC, OUTC], BF16, tag="woT")
            for k in range(KC):
                nc.sync.dma_start(woT_sb[:, k, :], woT[ts(k, P), :])

            ao0 = [None] * KC  # first-half ao tiles, prefetched per head
            with (
                tc.tile_pool(name="p2", bufs=2) as p2,
                tc.tile_pool(name="p2s", bufs=4) as p2s,
                tc.tile_pool(name="paob", bufs=12) as paob,
                tc.tile_pool(name="psum_s", bufs=2, space="PSUM") as psum_s_pool,
                tc.tile_pool(name="psum_o", bufs=2, space="PSUM") as psum_o_pool,
                tc.tile_pool(name="psum_t2", bufs=2, space="PSUM") as psum_t2,
            ):
                def emit_scores(h, j):
                    nkb = 4 * (j + 1)
                    attn = p2.tile([P, NT, QSPAN], BF16, tag="attn",
                                   name=f"attn_{h}_{j}")
                    for ip in range(nkb // 2):
                        i = 2 * ip
                        ps = psum_s_pool.tile([P, 2, QSPAN], F32, tag="ps")
                        for u in range(2):
                            nc.tensor.matmul(
                                ps[:, u, :],
                                qkT[:, QH, ts(i + u, P)],
                                qkT[:, h, ds(j * QSPAN, QSPAN)],
                                start=True, stop=True,
                            )
                        # one exp over both blocks (amortize ACT fixed cost)
                        _ei = nc.scalar.activation(
                            attn[:, i : i + 2, :], ps[:], ACTF.Exp, scale=SCALE
                        )
                        if h == 0 and aps.get("last_sqrt") is not None:
                            # keep early exps ordered after the last LN sqrt so
                            # the scheduler can't thrash the ACT table set
                            _bass_rust.add_dep_helper(
                                _ei.ins, aps["last_sqrt"], sync=True,
                                reason="ACT table-set ordering",
                            )
                        r = i - 4 * j
                        if r >= 0:
                            # diagonal pair: one masking mul over both blocks
                            nc.vector.tensor_mul(
                                attn[:, i : i + 2, :],
                                attn[:, i : i + 2, :],
                                masks_sb[:, r : r + 2, :],
                            )
                    return attn

                def emit_pv_mm(h, j, attn):
                    aobs = []
                    for q4 in range(4):
                        qb = 4 * j + q4
                        po = psum_o_pool.tile([P, HD + 1], F32, tag="po")
                        for i in range(qb + 1):
                            nc.tensor.matmul(
                                po[:],
                                attn[:, i, ts(q4, P)],
                                vaug[:, i, :],
                                start=(i == 0), stop=(i == qb),
                            )
                        recip = p2s.tile([P, 1], F32, tag="recip")
                        nc.vector.reciprocal(recip[:], po[:, HD : HD + 1])
                        aob = paob.tile([P, HD], BF16, tag="aob",
                                        name=f"aob_{h}_{qb}")
                        nc.vector.tensor_scalar_mul(aob[:], po[:, 0:HD], recip[:])
                        aobs.append(aob)
                    return aobs

                def emit_tr(h, j, aobs, aoTh):
                    for q4 in range(4):
                        qb = 4 * j + q4
                        pt2 = psum_t2.tile([P, P], BF16, tag="pt2")
                        nc.tensor.transpose(pt2[:], aobs[q4][:], ident_sb[:])
                        nc.vector.tensor_copy(aoTh[:, ts(qb, P)], pt2[:])
                    if j == NQS - 1:
                        nc.sync.dma_start(ag_in[ts(h, P), :], aoTh[:])
                        _post_head(h)

                def _post_head(h):
                    if aps.get("no_collective"):
                        # split by token halves so the first-half ao prefetch
                        # (all that phase-3 group 0 needs) lands early
                        for th in range(2):
                            nc.sync.dma_start(
                                ag_out[h].rearrange("(r p) t -> r p t", p=P)[
                                    :, :, ds(th * (T // 2), T // 2)
                                ],
                                ag_in.rearrange("(a p) t -> a p t", p=P)[
                                    ds(h, 1), :, ds(th * (T // 2), T // 2)
                                ].to_broadcast([NCORES, P, T // 2]),
                            )
                    else:
                        nc.gpsimd.collective_compute(
                            "AllGather",
                            ALU.bypass,
                            replica_groups=[list(range(NCORES))],
                            ins=[ag_in[ts(h, P), :]],
                            outs=[ag_out[h]],
                        )
                    # prefetch this head's first-half ao tiles for phase 3
                    for r in range(NCORES):
                        k = h * NCORES + r
                        a = p3.tile([P, T // 2], BF16, tag="ao", name=f"ao_0_{k}")
                        nc.sync.dma_start(a[:], ag_out[h, ts(r, P), ds(0, T // 2)])
                        ao0[k] = a

                # software pipeline: scores(j) | pv(j-2) | transpose(j-3)
                from collections import deque

                pv_q = deque()   # (h, j, attn)
                tr_q = deque()   # (h, j, aobs, aoTh)
                aoThs = {}
                spans = [(h, j) for h in range(QH) for j in range(NQS)]

                def step_pv():
                    ph, pj, pattn = pv_q.popleft()
                    tr_q.append((ph, pj, emit_pv_mm(ph, pj, pattn), aoThs[ph]))

                def step_tr():
                    emit_tr(*tr_q.popleft())

                for h, j in spans:
                    if j == 0:
                        aoThs[h] = paoT.tile(
                            [P, T], BF16, tag="aoTh", name=f"aoT_{h}"
                        )
                    attn = emit_scores(h, j)
                    pv_q.append((h, j, attn))
                    if len(pv_q) > 1:
                        step_pv()
                    if len(tr_q) > 1:
                        step_tr()
                while pv_q:
                    step_pv()
                    while len(tr_q) > 1:
                        step_tr()
                while tr_q:
                    step_tr()

            # ---------------- Phase 3: output projection ----------------
            with tc.tile_pool(name="psum3", bufs=8, space="PSUM") as psum3:
                ao1 = [None] * KC
                for th in range(2):  # token halves
                    for cbg in range(2):  # 2 col-block groups -> evac overlap
                        pos = [
                            psum3.tile([P, 512], F32, tag="po3",
                                       name=f"po3_{th}_{cbg}_{i}")
                            for i in range(4)
                        ]
                        for k in range(KC):
                            h, r = divmod(k, NCORES)
                            if th == 0:
                                a = ao0[k]
                            elif cbg == 0:
                                a = p3.tile([P, T // 2], BF16, tag="ao",
                                            name=f"ao_1_{k}")
                                nc.sync.dma_start(
                                    a[:], ag_out[h, ts(r, P), ds(T // 2, T // 2)]
                                )
                                ao1[k] = a
                            else:
                                a = ao1[k]
                            for cc in range(2):
                                cb = cbg * 2 + cc
                                for s2 in range(2):
                                    nc.tensor.matmul(
                                        pos[cc * 2 + s2][:],
                                        woT_sb[:, k, ts(cb, P)],
                                        a[:, ts(s2, 512)],
                                        start=(k == 0), stop=(k == KC - 1),
                                    )
                        for cc in range(2):
                            cb = cbg * 2 + cc
                            for s2 in range(2):
                                ob = p3o.tile(
                                    [P, 512], BF16, tag="ob",
                                    name=f"ob_{th}_{cb}_{s2}"
                                )
                                # split evacuation across DVE and ACT so the
                                # final drain isn't serial on one engine
                                if s2 == 0:
                                    nc.vector.tensor_copy(ob[:], pos[cc * 2 + s2][:])
                                else:
                                    nc.scalar.copy(ob[:], pos[cc * 2 + s2][:])
                                nc.sync.dma_start(
                                    outT[ts(cb, P), ds(th * (T // 2) + s2 * 512, 512)],
                                    ob[:],
                                )


def _build_program(no_collective=False, reps=1, wb_trivial=True):
    nc = bacc.Bacc(
        "TRN2",
        target_bir_lowering=False,
        debug=False,
        enable_asserts=True,
        num_devices=1 if no_collective else NCORES,
    )
    aps = {
        "xT8": nc.dram_tensor(
            "xT8", [NT, 2, P, KC, P], FP8, kind="ExternalInput"
        ).ap(),
        "wqkvT8": nc.dram_tensor(
            "wqkvT8", [DIM, 2, FEAT], FP8, kind="ExternalInput"
        ).ap(),
        "woT": nc.dram_tensor("woT", [NH * HD, OUTC], BF16, kind="ExternalInput").ap(),
        "ropeP": nc.dram_tensor(
            "ropeP", [P, NT, 2, HALF], F32, kind="ExternalInput"
        ).ap(),
        "lnwb": nc.dram_tensor("lnwb", [P, 2, 2, HD], F32, kind="ExternalInput").ap(),
        "masks": nc.dram_tensor("masks", [P, 4, QSPAN], BF16, kind="ExternalInput").ap(),
        "ident": nc.dram_tensor("ident", [P, P], BF16, kind="ExternalInput").ap(),
        "ag_in": nc.dram_tensor("ag_in", [QH * HD, T], BF16).ap(),
        "ag_out": nc.dram_tensor(
            "ag_out", [QH, NCORES * P, T], BF16, addr_space="Shared"
        ).ap(),
        "outT": nc.dram_tensor("outT", [OUTC, T], BF16, kind="ExternalOutput").ap(),
    }
    aps["no_collective"] = no_collective
    aps["wb_trivial"] = wb_trivial
    with tile.TileContext(nc) as tc:
        aps["tc"] = tc
        for _rep in range(reps):
            _build_body(nc, aps)
    nc.compile()
    return nc


def get_program(wb_trivial=True):
    key = ("nc", wb_trivial)
    if key not in _PROGRAM_CACHE:
        _PROGRAM_CACHE[key] = _build_program(wb_trivial=wb_trivial)
    return _PROGRAM_CACHE[key]


def _rope_tables():
    """cos/sin tables computed exactly like the reference (jax fp32 on cpu)."""
    try:
        import jax

        cpu = jax.devices("cpu")[0]
        with jax.default_device(cpu):
            import jax.numpy as jnp

            inv_freq = 1.0 / (
                THETA ** (jnp.arange(HALF, dtype=jnp.float32) * 2.0 / HD)
            )
            pos = jnp.arange(T, dtype=jnp.float32)
            ang = pos[:, None] * inv_freq[None, :]
            cos = np.asarray(jnp.cos(ang), dtype=np.float32)
            sin = np.asarray(jnp.sin(ang), dtype=np.float32)
    except Exception:
        inv_freq = (
            1.0 / (THETA ** (np.arange(HALF, dtype=np.float32) * 2.0 / HD))
        ).astype(np.float32)
        ang = np.arange(T, dtype=np.float32)[:, None] * inv_freq[None, :]
        cos = np.cos(ang).astype(np.float32)
        sin = np.sin(ang).astype(np.float32)
    return cos, sin


def _make_const_inputs(q_ln_w, q_ln_b, k_ln_w, k_ln_b):
    cos, sin = _rope_tables()  # [T, HALF] f32
    ropeP = np.zeros((P, NT, 2, HALF), np.float32)
    ropeP[:, :, 0] = cos.reshape(NT, P, HALF).transpose(1, 0, 2)
    ropeP[:, :, 1] = sin.reshape(NT, P, HALF).transpose(1, 0, 2)

    lnwb = np.zeros((P, 2, 2, HD), np.float32)
    lnwb[:, 0, 0] = np.asarray(q_ln_w, np.float32)[None, :]
    lnwb[:, 0, 1] = np.asarray(q_ln_b, np.float32)[None, :]
    lnwb[:, 1, 0] = np.asarray(k_ln_w, np.float32)[None, :]
    lnwb[:, 1, 1] = np.asarray(k_ln_b, np.float32)[None, :]

    f = np.arange(QSPAN)[None, None, :]
    r = np.arange(4)[None, :, None]
    p = np.arange(P)[:, None, None]
    masks = (f >= 128 * r + p).astype(ml_dtypes.bfloat16)  # [P, 4, QSPAN]
    ident = np.eye(P, dtype=ml_dtypes.bfloat16)
    return ropeP, lnwb, masks, ident


# phase-3 lhsT rows are ordered (h, r, d) = head-of-rank h, rank r; the ao
# feature order is (global head g = 4r+h, d). Permute woT rows to match.
_WOT_PERM = np.empty(NH * HD, np.int64)
for _h in range(QH):
    for _r in range(NCORES):
        _j = (_h * NCORES + _r) * HD
        _g = (4 * _r + _h) * HD
        _WOT_PERM[_j : _j + HD] = np.arange(_g, _g + HD)


def _split8(x):
    hi = x.astype(ml_dtypes.float8_e4m3)
    lo = (x - hi.astype(np.float32)).astype(ml_dtypes.float8_e4m3)
    return hi, lo


def make_in_maps(inputs):
    x = np.asarray(inputs["x"], dtype=ml_dtypes.bfloat16).astype(np.float32)
    wqkv = np.asarray(inputs["wqkv"], dtype=ml_dtypes.bfloat16).astype(np.float32)
    wo = np.asarray(inputs["wo"], dtype=ml_dtypes.bfloat16)
    q_ln_w = np.asarray(inputs["q_ln_w"], np.float32)
    q_ln_b = np.asarray(inputs["q_ln_b"], np.float32)
    k_ln_w = np.asarray(inputs["k_ln_w"], np.float32)
    k_ln_b = np.asarray(inputs["k_ln_b"], np.float32)

    ropeP, lnwb, masks, ident = _make_const_inputs(q_ln_w, q_ln_b, k_ln_w, k_ln_b)
    xh, xl = _split8(SX * x)

    def _tile_x(a):
        return a.reshape(NT, P, KC, P).transpose(0, 3, 2, 1)

    xT8 = np.ascontiguousarray(np.stack([_tile_x(xl), _tile_x(xh)], axis=1))

    in_maps = []
    for c in range(NCORES):
        qrows = wqkv[c * QH * HD : (c + 1) * QH * HD]
        krows = wqkv[NH * HD + c * HD : NH * HD + (c + 1) * HD]
        vrows = wqkv[(NH + NKV) * HD + c * HD : (NH + NKV) * HD + (c + 1) * HD]
        wq_c = np.concatenate([qrows, krows, vrows], axis=0).T
        wh, wl = _split8(SW * wq_c)
        wqkvT8_c = np.ascontiguousarray(np.stack([wh, wl], axis=1))
        woT_c = np.ascontiguousarray(
            wo[c * OUTC : (c + 1) * OUTC, :].T[_WOT_PERM, :]
        )
        in_maps.append(
            {
                "xT8": xT8,
                "wqkvT8": wqkvT8_c,
                "woT": woT_c,
                "ropeP": ropeP,
                "lnwb": lnwb,
                "masks": masks,
                "ident": ident,
            }
        )
    return in_maps


def _wb_trivial(inputs):
    return bool(
        np.all(np.asarray(inputs["q_ln_w"], np.float32) == 1.0)
        and np.all(np.asarray(inputs["k_ln_w"], np.float32) == 1.0)
        and np.all(np.asarray(inputs["q_ln_b"], np.float32) == 0.0)
        and np.all(np.asarray(inputs["k_ln_b"], np.float32) == 0.0)
    )


def kernel(**inputs):
    nc = get_program(wb_trivial=_wb_trivial(inputs))
    in_maps = make_in_maps(inputs)
    res = run_bass_kernel_spmd(nc, in_maps, list(range(NCORES)))
    outT_full = np.concatenate(
        [np.asarray(res.results[c]["outT"]) for c in range(NCORES)], axis=0
    )
    return np.ascontiguousarray(outT_full.T).astype(ml_dtypes.bfloat16)


if __name__ == "__main__":
    nc = get_program()
    print("program built ok")

